# revision 18
# baseline (speedup 1.0000x reference)
"""Trainium2 Bass kernel for CRF Viterbi decode (nn_CRF).

Problem (hardcoded): x[64, 512, 1024] @ kernel[1024, 128] + bias -> logits
[B, T, U]; boundary energies added on first/last timestep; Viterbi decode
with transition matrix chain_kernel[128, 128]; returns tags as float32.

Strategy
--------
Data-parallel over 8 NeuronCores: 8 batch elements per core.

Device (per core):
  1. logits matmul: x^T (pre-transposed on host to [D, (t, b)] layout)
     against kernel tiles, accumulated in PSUM over K=1024, evacuated by
     the scalar engine (bias fused) into an SBUF-resident logitsT[u, (t,b)].
  2. Viterbi forward scan (511 serial steps), batches split into 2 groups
     of 4 that pipeline across engines. Per step and group:
       - PE broadcasts v rows into PSUM (K=1 matmuls against a ones row)
         and accumulates trans[i, j] on top via an identity-tiled matmul
         (exact fp32: each PSUM element sees v[b,i] + trans[i,j] as a
         single fp32 add).
       - DVE does a segmented reduce-max over i -> max values [u, 4].
       - DVE adds logit_t -> new v, written into an SBUF vhist tile.
       - PE transposes the new v column block; ACT evacuates it to SBUF
         as rows for the next step's broadcast matmuls.
     Only max VALUES are kept (no argmax on device) - backpointers are
     reconstructed exactly on the host from vhist, since fp32 add/max
     here are bit-exact reproducible.
  3. vhist is DMAd out chunk-by-chunk as the scan progresses.

Host: shard/pre-transpose inputs, run SPMD on cores 0-7, then backtrace:
  tag_t = argmax_i(v_t[b, i] + trans[i, tag_{t+1}]) - bit-identical to the
  backpointers the device forward pass implies.
"""

import os

import numpy as np

import concourse.bass as bass
import concourse.mybir as mybir
from concourse.tile import TileContext
from concourse.bass_utils import run_bass_kernel_spmd

F32 = mybir.dt.float32

# Problem constants
B, T, D, U = 64, 512, 1024, 128
NCORES = 8
BL = B // NCORES           # batches per core (8)
NG = 2                     # batch pipeline groups
GB = BL // NG              # batches per group (4)

last_results = None        # BassKernelResults of the most recent kernel() run


def split_multi_waits(nc):
    """The walrus build in this container encodes at most ONE sync wait per
    compute/DMA instruction ("Too many sync wait commands" otherwise). Hoist
    all but the last wait of any multi-wait instruction onto standalone
    same-engine EventSemaphore ops placed immediately before it (engine
    queues execute in order, so semantics are preserved)."""
    for f in nc.m.functions:
        for blk in f.blocks:
            new_insts = []
            changed = False
            for inst in blk.instructions:
                si = inst.sync_info
                if si is not None and len(si.on_wait) > 1:
                    waits = list(si.on_wait)
                    for k, w in enumerate(waits[:-1]):
                        new_insts.append(mybir.InstEventSemaphore(
                            name=f"{inst.name}-sw{k}",
                            engine=inst.engine,
                            ins=[], outs=[],
                            sync_info=mybir.SyncInfo(on_wait=[w], on_update=[]),
                        ))
                    inst.sync_info = mybir.SyncInfo(
                        on_wait=[waits[-1]], on_update=list(si.on_update))
                    changed = True
                new_insts.append(inst)
            if changed:
                blk.instructions = new_insts
    return nc


def build_program(t_steps=T, d_dim=D, split_waits=True, scan_reps=1,
                  loop_reps=None, warm=0, mode=None):
    """mode: 'base' = previous kernel; 'a' = DVE stt logit-add (no ACT);
    'ab' = 'a' + K=32 tile-position-concurrent PE matmuls for the
    broadcast/trans rounds."""
    if mode is None:
        mode = os.environ.get("CRF_MODE", "base")
    nt = t_steps * BL                       # columns in (t, b) layout
    ch = min(512, nt)                       # DMA/matmul chunk width
    nch = nt // ch
    kblocks = d_dim // 128

    nc = bass.Bass(trn_type="TRN2")

    xdt = nc.dram_tensor("xdt", [d_dim, nt], F32, kind="ExternalInput")
    ker = nc.dram_tensor("ker", [d_dim, U], F32, kind="ExternalInput")
    translhs = nc.dram_tensor("translhs", [U, U], F32, kind="ExternalInput")
    ident = nc.dram_tensor("ident", [U, U], F32, kind="ExternalInput")
    ident32 = nc.dram_tensor("ident32", [U, 32], F32, kind="ExternalInput")
    lbv = nc.dram_tensor("lbv", [U, 1], F32, kind="ExternalInput")
    rbv = nc.dram_tensor("rbv", [U, 1], F32, kind="ExternalInput")
    biasrow = nc.dram_tensor("biasrow", [1, U], F32, kind="ExternalInput")
    onesrow = nc.dram_tensor("onesrow", [1, 512], F32, kind="ExternalInput")
    vout = nc.dram_tensor("vout", [U, nt], F32, kind="ExternalOutput")

    with TileContext(nc) as tc:
        with (
            tc.tile_pool(name="const", bufs=1) as cpool,
            tc.tile_pool(name="xp", bufs=10) as xpool,
            tc.tile_pool(name="big", bufs=1) as bigpool,
            tc.tile_pool(name="mx", bufs=3) as mxpool,
            tc.tile_pool(name="mmps", bufs=2, space="PSUM") as mmpool,
            tc.tile_pool(name="scps", bufs=2, space="PSUM") as scpool,
            tc.tile_pool(name="wmps", bufs=1, space="PSUM") as wmpool,
        ):
            # ---- constants into SBUF ----
            ker_sb = []
            for kb in range(kblocks):
                kt = cpool.tile([128, U], F32, tag=f"ker{kb}")
                nc.sync.dma_start(out=kt[:, :], in_=ker[kb * 128:(kb + 1) * 128, :])
                ker_sb.append(kt)
            trans_sb = cpool.tile([U, U], F32, tag="trans")
            nc.sync.dma_start(out=trans_sb[:, :], in_=translhs[:, :])
            ident_sb = cpool.tile([U, U], F32, tag="ident")
            nc.sync.dma_start(out=ident_sb[:, :], in_=ident[:, :])
            ident32_sb = cpool.tile([U, 32], F32, tag="ident32")
            nc.sync.dma_start(out=ident32_sb[:, :], in_=ident32[:, :])
            lb_sb = cpool.tile([U, 1], F32, tag="lb")
            nc.sync.dma_start(out=lb_sb[:, :], in_=lbv[:, :])
            rb_sb = cpool.tile([U, 1], F32, tag="rb")
            nc.sync.dma_start(out=rb_sb[:, :], in_=rbv[:, :])
            biasrow_sb = cpool.tile([1, U], F32, tag="biasrow")
            nc.sync.dma_start(out=biasrow_sb[:, :], in_=biasrow[:, :])
            onesrow_sb = cpool.tile([1, 512], F32, tag="onesrow")
            nc.sync.dma_start(out=onesrow_sb[:, :], in_=onesrow[:, :])

            logitsT = bigpool.tile([U, nt], F32, tag="logitsT")
            # per-group v history (decoupled so the two batch-group
            # pipelines never serialize on tile dependency tracking);
            # group g columns: t * GB + bb
            vh = [bigpool.tile([U, nt // NG], F32, tag=f"vh{g}",
                                name=f"vh{g}")
                  for g in range(NG)]

            # ---- phase 1: logits = kernel.T @ x (+bias) ----
            for c in range(nch):
                ps = mmpool.tile([128, ch], F32, tag="mm")
                for kb in range(kblocks):
                    xt = xpool.tile([128, ch], F32, tag="x")
                    nc.sync.dma_start(
                        out=xt[:, :],
                        in_=xdt[kb * 128:(kb + 1) * 128, c * ch:(c + 1) * ch],
                    )
                    nc.tensor.matmul(
                        out=ps[:, :], lhsT=ker_sb[kb][:, :], rhs=xt[:, :],
                        start=(kb == 0), stop=False,
                    )
                nc.tensor.matmul(
                    out=ps[:, :], lhsT=biasrow_sb[0:1, :],
                    rhs=onesrow_sb[0:1, 0:ch], start=False, stop=True,
                )
                nc.scalar.copy(
                    out=logitsT[:, c * ch:(c + 1) * ch], in_=ps[:, :],
                )

            # right boundary folded into the last timestep's logits
            nc.vector.tensor_scalar_add(
                out=logitsT[:, (t_steps - 1) * BL:],
                in0=logitsT[:, (t_steps - 1) * BL:],
                scalar1=rb_sb[:, 0:1],
            )
            # ---- phase 2: Viterbi forward scan ----
            # v broadcast across partitions: matmul with the previous v
            # column as stationary operand, broadcast along its free dim
            # (step-0 AP), against an identity moving operand:
            #   out[p, i] = sum_k v[k] * I[k, i] = v[i]  for every p.
            # scan_reps > 1 repeats the whole scan (for differential
            # wall-clock timing); results are identical each rep.
            steps_per_chunk = ch // BL
            gch = steps_per_chunk * GB          # per-group chunk width
            import contextlib
            rep_ctx = (tc.For_i(0, loop_reps, 1) if loop_reps
                       else contextlib.nullcontext())
            if mode.startswith("c"):
                # ---- mode c: 6 PE/DVE batches (2 groups of 3) + 2 ACT/Pool
                # batches. vout layout: [vhE0 (3T) | vhE1 (3T) | vhA (2T)].
                import concourse.bass_isa as bass_isa
                NEG, EGB = 2, 3                   # E groups, batches/group
                vhE = [bigpool.tile([U, t_steps * EGB], F32, tag=f"vhe{g}",
                                    name=f"vhe{g}") for g in range(NEG)]
                vhA = bigpool.tile([U, t_steps * 2], F32, tag="vha",
                                   name="vha")
                areduce = os.environ.get("CRF_AREDUCE", "pallred")
                if areduce == "pallred":
                    from concourse import library_config
                    nc.gpsimd.load_library(library_config.mlp)
                with (
                    tc.tile_pool(name="sa", bufs=2) as sapool,
                    tc.tile_pool(name="ma", bufs=2) as mapool,
                    tc.tile_pool(name="mc", bufs=2, space="PSUM") as mcpool,
                ):
                  with rep_ctx:
                    for _rep in range(scan_reps):
                      for g in range(NEG):
                          nc.vector.tensor_scalar_add(
                              out=vhE[g][:, 0:EGB],
                              in0=logitsT[:, g * EGB:(g + 1) * EGB],
                              scalar1=lb_sb[:, 0:1])
                      nc.vector.tensor_scalar_add(
                          out=vhA[:, 0:2], in0=logitsT[:, 6:8],
                          scalar1=lb_sb[:, 0:1])
                      for t in range(1, t_steps):
                          for g in range(NEG):
                              lcols0 = t * BL + g * EGB
                              vcols0 = t * EGB
                              pcol0 = (t - 1) * EGB
                              sc = scpool.tile([128, EGB * U], F32,
                                               tag=f"sc{g}")
                              if mode == "cT":
                                  for bb in range(EGB):
                                      vcol = vhE[g][:, pcol0 + bb:
                                                    pcol0 + bb + 1]
                                      nc.tensor.matmul(
                                          out=sc[:, bb * U:(bb + 1) * U],
                                          lhsT=vcol.broadcast_to([U, U]),
                                          rhs=ident_sb[:, :],
                                          start=(bb == 0), stop=False,
                                          skip_group_check=True,
                                          is_transpose=True)
                                  for bb in range(EGB):
                                      nc.tensor.matmul(
                                          out=sc[:, bb * U:(bb + 1) * U],
                                          lhsT=trans_sb[:, :],
                                          rhs=ident_sb[:, :],
                                          start=False, stop=(bb == EGB - 1),
                                          skip_group_check=True,
                                          is_transpose=True)
                              else:
                                  for bb in range(EGB):
                                      vcol = vhE[g][:, pcol0 + bb:
                                                    pcol0 + bb + 1]
                                      for blk in range(4):
                                          nc.tensor.matmul(
                                              out=sc[:, bb * U + 32 * blk:
                                                     bb * U + 32 * blk + 32],
                                              lhsT=vcol[32 * blk:
                                                        32 * blk + 32, :]
                                                  .broadcast_to([32, U]),
                                              rhs=ident32_sb[32 * blk:
                                                             32 * blk + 32, :],
                                              start=(bb == 0 and blk == 0),
                                              stop=False,
                                              skip_group_check=True,
                                              tile_position=(32 * blk, 0))
                                  for bb in range(EGB):
                                      for blk in range(4):
                                          nc.tensor.matmul(
                                              out=sc[:, bb * U + 32 * blk:
                                                     bb * U + 32 * blk + 32],
                                              lhsT=trans_sb[32 * blk:
                                                            32 * blk + 32, :],
                                              rhs=ident32_sb[32 * blk:
                                                             32 * blk + 32, :],
                                              start=False,
                                              stop=(bb == EGB - 1 and
                                                    blk == 3),
                                              skip_group_check=True,
                                              tile_position=(32 * blk, 0))
                              mx = mxpool.tile([U, EGB], F32, tag=f"mx{g}")
                              nc.vector.tensor_reduce(
                                  out=mx[:, :],
                                  in_=sc[:, :].rearrange("p (b i) -> p b i",
                                                         i=U),
                                  axis=mybir.AxisListType.X,
                                  op=mybir.AluOpType.max)
                              nc.vector.tensor_add(
                                  out=vhE[g][:, vcols0:vcols0 + EGB],
                                  in0=mx[:, :],
                                  in1=logitsT[:, lcols0:lcols0 + EGB])
                          # A route: scores[i, j] built by ACT (bias = prev
                          # v col), partition-max on Pool, PE transposes the
                          # result row back to a column, DVE adds the logit.
                          mc = mcpool.tile([U, 2], F32, tag="mc")
                          for ai in range(2):
                              sa = sapool.tile([U, U], F32, tag=f"sa{ai}")
                              nc.scalar.activation(
                                  out=sa[:, :], in_=trans_sb[:, :],
                                  func=mybir.ActivationFunctionType.Identity,
                                  bias=vhA[:, (t - 1) * 2 + ai:
                                           (t - 1) * 2 + ai + 1])
                              if areduce == "pallred":
                                  ma = mapool.tile([U, U], F32, tag=f"ma{ai}")
                                  nc.gpsimd.partition_all_reduce(
                                      out_ap=ma[:, :], in_ap=sa[:, :],
                                      channels=U,
                                      reduce_op=bass_isa.ReduceOp.max)
                                  marow = ma[0:1, :]
                              else:
                                  ma = mapool.tile([1, U], F32, tag=f"ma{ai}")
                                  nc.gpsimd.tensor_reduce(
                                      out=ma[:, :], in_=sa[:, :],
                                      axis=mybir.AxisListType.C,
                                      op=mybir.AluOpType.max)
                                  marow = ma[:, :]
                              nc.tensor.matmul(
                                  out=mc[:, ai:ai + 1], lhsT=marow,
                                  rhs=ident_sb[0:1, 0:1], start=True,
                                  stop=True, skip_group_check=True,
                                  is_transpose=True)
                          nc.vector.tensor_add(
                              out=vhA[:, t * 2:t * 2 + 2], in0=mc[:, 0:2],
                              in1=logitsT[:, t * BL + 6:t * BL + 8])
                          if (t + 1) % steps_per_chunk == 0:
                              cc = (t + 1) // steps_per_chunk - 1
                              spc = steps_per_chunk
                              for g in range(NEG):
                                  nc.sync.dma_start(
                                      out=vout[:, g * (t_steps * EGB) +
                                               cc * spc * EGB:
                                               g * (t_steps * EGB) +
                                               (cc + 1) * spc * EGB],
                                      in_=vhE[g][:, cc * spc * EGB:
                                                 (cc + 1) * spc * EGB])
                              nc.sync.dma_start(
                                  out=vout[:, 6 * t_steps + cc * spc * 2:
                                           6 * t_steps + (cc + 1) * spc * 2],
                                  in_=vhA[:, cc * spc * 2:(cc + 1) * spc * 2])
            if mode.startswith("c"):
                # c-scan already emitted above; skip the default scan body
                rep_ctx = contextlib.nullcontext()
                scan_iter = []
            else:
                scan_iter = range(scan_reps)
            with rep_ctx:
             for _rep in scan_iter:
              # v_0 = logits_0 + left boundary
              for g in range(NG):
                nc.vector.tensor_scalar_add(
                    out=vh[g][:, 0:GB], in0=logitsT[:, g * GB:(g + 1) * GB],
                    scalar1=lb_sb[:, 0:1],
                )
              for t in range(1, t_steps):
                  for g in range(NG):
                      lcols0 = t * BL + g * GB    # logitsT columns
                      vcols0 = t * GB             # vh[g] columns
                      pcol0 = (t - 1) * GB
                      sc = scpool.tile([128, GB * U], F32, tag=f"sc{g}")
                      if mode in ("b", "ab", "a2b", "a3b"):
                          # scores via K=32 row-group-tiled matmuls: the 4
                          # tiles of a round occupy distinct PE row groups
                          # (tile_position) and run concurrently. Broadcast
                          # round: out[p, i in blk] = sum_k v[32B+k] I[k,i]
                          # = v[i]; trans round accumulates trans[i, p].
                          # start=True only on the tile's very first matmul:
                          # start clears has_written for the whole PSUM bank,
                          # so a later start would orphan earlier blocks'
                          # writes (their accumulation step then overwrites).
                          for bb in range(GB):
                              vcol = vh[g][:, pcol0 + bb:pcol0 + bb + 1]
                              for blk in range(4):
                                  nc.tensor.matmul(
                                      out=sc[:, bb * U + 32 * blk:
                                             bb * U + 32 * blk + 32],
                                      lhsT=vcol[32 * blk:32 * blk + 32, :]
                                          .broadcast_to([32, U]),
                                      rhs=ident32_sb[32 * blk:32 * blk + 32, :],
                                      start=(bb == 0 and blk == 0), stop=False,
                                      skip_group_check=True,
                                      tile_position=(32 * blk, 0),
                                  )
                          for bb in range(GB):
                              for blk in range(4):
                                  nc.tensor.matmul(
                                      out=sc[:, bb * U + 32 * blk:
                                             bb * U + 32 * blk + 32],
                                      lhsT=trans_sb[32 * blk:32 * blk + 32, :],
                                      rhs=ident32_sb[32 * blk:32 * blk + 32, :],
                                      start=False,
                                      stop=(bb == GB - 1 and blk == 3),
                                      skip_group_check=True,
                                      tile_position=(32 * blk, 0),
                                  )
                      else:
                          for bb in range(GB):
                              vcol = vh[g][:, pcol0 + bb:pcol0 + bb + 1]
                              nc.tensor.matmul(
                                  out=sc[:, bb * U:(bb + 1) * U],
                                  lhsT=vcol.broadcast_to([U, U]),
                                  rhs=ident_sb[:, :],
                                  start=(bb == 0), stop=False,
                                  skip_group_check=True, is_transpose=True,
                              )
                          for bb in range(GB):
                              nc.tensor.matmul(
                                  out=sc[:, bb * U:(bb + 1) * U],
                                  lhsT=trans_sb[:, :], rhs=ident_sb[:, :],
                                  start=False, stop=(bb == GB - 1),
                                  skip_group_check=True, is_transpose=True,
                              )
                      # optional HAM-warming filler: keeps the PE p-state
                      # hot across the per-step stall waiting for v(t)
                      for _w in range(warm):
                          wt = wmpool.tile([U, U], F32, tag="warm")
                          nc.tensor.matmul(
                              out=wt[:, :], lhsT=ident_sb[:, :],
                              rhs=ident_sb[:, :], start=True, stop=True,
                              skip_group_check=True, is_transpose=True,
                          )
                      mx = mxpool.tile([U, GB], F32, tag=f"mx{g}")
                      nc.vector.tensor_reduce(
                          out=mx[:, :],
                          in_=sc[:, :].rearrange("p (b i) -> p b i", i=U),
                          axis=mybir.AxisListType.X, op=mybir.AluOpType.max,
                      )
                      if mode in ("a", "ab"):
                          # v = mx + logit in one grouped DVE op (the ACT
                          # per-column path costs ~190 ns/instruction)
                          nc.vector.scalar_tensor_tensor(
                              out=vh[g][:, vcols0:vcols0 + GB],
                              in0=mx[:, :], scalar=1.0,
                              in1=logitsT[:, lcols0:lcols0 + GB],
                              op0=mybir.AluOpType.mult,
                              op1=mybir.AluOpType.add,
                          )
                      elif mode in ("a2", "a2b"):
                          nc.vector.tensor_add(
                              out=vh[g][:, vcols0:vcols0 + GB],
                              in0=mx[:, :],
                              in1=logitsT[:, lcols0:lcols0 + GB],
                          )
                      elif mode in ("a3", "a3b"):
                          # logit add on the otherwise-idle Pool engine
                          nc.gpsimd.tensor_add(
                              out=vh[g][:, vcols0:vcols0 + GB],
                              in0=mx[:, :],
                              in1=logitsT[:, lcols0:lcols0 + GB],
                          )
                      else:
                          for bb in range(GB):
                              nc.scalar.activation(
                                  out=vh[g][:, vcols0 + bb:vcols0 + bb + 1],
                                  in_=mx[:, bb:bb + 1],
                                  func=mybir.ActivationFunctionType.Identity,
                                  bias=logitsT[:, lcols0 + bb:lcols0 + bb + 1],
                              )
                  if (t + 1) % steps_per_chunk == 0:
                      c = (t + 1) // steps_per_chunk - 1
                      for g in range(NG):
                          nc.sync.dma_start(
                              out=vout[:, g * (nt // NG) + c * gch:
                                       g * (nt // NG) + (c + 1) * gch],
                              in_=vh[g][:, c * gch:(c + 1) * gch],
                          )
    return split_multi_waits(nc) if split_waits else nc


def make_in_map(x_core, ker, bias, trans, lb, rb, t_steps=T, d_dim=D):
    """x_core: [BL, t_steps, d_dim] float32."""
    nt = t_steps * BL
    xdt = np.ascontiguousarray(x_core.transpose(2, 1, 0)).reshape(d_dim, nt)
    return {
        "xdt": xdt.astype(np.float32),
        "ker": np.ascontiguousarray(ker, dtype=np.float32),
        "biasrow": np.ascontiguousarray(bias, dtype=np.float32).reshape(1, U),
        "onesrow": np.ones((1, 512), dtype=np.float32),
        "translhs": np.ascontiguousarray(trans, dtype=np.float32),
        "ident": np.eye(U, dtype=np.float32),
        "ident32": np.tile(np.eye(32, dtype=np.float32), (U // 32, 1)),
        "lbv": np.ascontiguousarray(lb, dtype=np.float32).reshape(U, 1),
        "rbv": np.ascontiguousarray(rb, dtype=np.float32).reshape(U, 1),
    }


def backtrace(v, trans):
    """v: [b, t, u] forward max values; trans: [u, u]. Returns int tags [b, t]."""
    nb, nt, nu = v.shape
    tags = np.zeros((nb, nt), dtype=np.int64)
    cur = np.argmax(v[:, -1, :], axis=1)
    tags[:, -1] = cur
    for t in range(nt - 2, -1, -1):
        scores = v[:, t, :] + trans[:, cur].T     # fp32, same as device order
        cur = np.argmax(scores, axis=1)
        tags[:, t] = cur
    return tags


def vout_to_v(vout_core, t_steps=T, mode=None):
    """Decode vout -> v [BL, t, U] for the given mode's layout."""
    if mode is None:
        mode = os.environ.get("CRF_MODE", "base")
    if mode.startswith("c"):
        # [vhE0 (t*3+bb) | vhE1 | vhA (t*2+ai)]
        v = np.empty((BL, t_steps, U), np.float32)
        e0 = vout_core[:, 0:3 * t_steps].reshape(U, t_steps, 3)
        e1 = vout_core[:, 3 * t_steps:6 * t_steps].reshape(U, t_steps, 3)
        av = vout_core[:, 6 * t_steps:8 * t_steps].reshape(U, t_steps, 2)
        v[0:3] = e0.transpose(2, 1, 0)
        v[3:6] = e1.transpose(2, 1, 0)
        v[6:8] = av.transpose(2, 1, 0)
        return v
    v = vout_core.reshape(U, NG, t_steps, GB)     # [u, g, t, bb]
    return np.ascontiguousarray(v.transpose(1, 3, 2, 0).reshape(BL, t_steps, U))


def kernel(x, kernel, bias, chain_kernel, left_boundary, right_boundary):
    x = np.asarray(x, dtype=np.float32)
    ker = np.asarray(kernel, dtype=np.float32)
    bias = np.asarray(bias, dtype=np.float32)
    trans = np.asarray(chain_kernel, dtype=np.float32)
    lb = np.asarray(left_boundary, dtype=np.float32)
    rb = np.asarray(right_boundary, dtype=np.float32)

    nc = build_program()
    in_maps = [
        make_in_map(x[c * BL:(c + 1) * BL], ker, bias, trans, lb, rb)
        for c in range(NCORES)
    ]
    kwargs = {}
    if os.environ.get("CRF_TRACE"):
        kwargs = {"trace": True, "tmpdir": os.environ.get("CRF_TRACE_DIR") or None}
    res = run_bass_kernel_spmd(nc, in_maps, core_ids=list(range(NCORES)), **kwargs)
    global last_results
    last_results = res
    v = np.concatenate(
        [vout_to_v(np.asarray(r["vout"])) for r in res.results], axis=0)
    tags = backtrace(v, trans)
    return tags.astype(np.float32)



# revision 19
# speedup vs baseline: 1.1698x; 1.1698x over previous
"""Trainium2 Bass kernel for CRF Viterbi decode (nn_CRF).

Problem (hardcoded): x[64, 512, 1024] @ kernel[1024, 128] + bias -> logits
[B, T, U]; boundary energies added on first/last timestep; Viterbi decode
with transition matrix chain_kernel[128, 128]; returns tags as float32.

Strategy
--------
Data-parallel over 8 NeuronCores: 8 batch elements per core.

Device (per core):
  1. logits matmul: x^T (pre-transposed on host to [D, (t, b)] layout)
     against kernel tiles, accumulated in PSUM over K=1024, evacuated by
     the scalar engine (bias fused) into an SBUF-resident logitsT[u, (t,b)].
  2. Viterbi forward scan (511 serial steps), batches split into 2 groups
     of 4 that pipeline across engines. Per step and group:
       - PE broadcasts v rows into PSUM (K=1 matmuls against a ones row)
         and accumulates trans[i, j] on top via an identity-tiled matmul
         (exact fp32: each PSUM element sees v[b,i] + trans[i,j] as a
         single fp32 add).
       - DVE does a segmented reduce-max over i -> max values [u, 4].
       - DVE adds logit_t -> new v, written into an SBUF vhist tile.
       - PE transposes the new v column block; ACT evacuates it to SBUF
         as rows for the next step's broadcast matmuls.
     Only max VALUES are kept (no argmax on device) - backpointers are
     reconstructed exactly on the host from vhist, since fp32 add/max
     here are bit-exact reproducible.
  3. vhist is DMAd out chunk-by-chunk as the scan progresses.

Host: shard/pre-transpose inputs, run SPMD on cores 0-7, then backtrace:
  tag_t = argmax_i(v_t[b, i] + trans[i, tag_{t+1}]) - bit-identical to the
  backpointers the device forward pass implies.
"""

import os

import numpy as np

import concourse.bass as bass
import concourse.mybir as mybir
from concourse.tile import TileContext
from concourse.bass_utils import run_bass_kernel_spmd

F32 = mybir.dt.float32

# Problem constants
B, T, D, U = 64, 512, 1024, 128
NCORES = 8
BL = B // NCORES           # batches per core (8)
NG = 2                     # batch pipeline groups
GB = BL // NG              # batches per group (4)

last_results = None        # BassKernelResults of the most recent kernel() run


def split_multi_waits(nc):
    """The walrus build in this container encodes at most ONE sync wait per
    compute/DMA instruction ("Too many sync wait commands" otherwise). Hoist
    all but the last wait of any multi-wait instruction onto standalone
    same-engine EventSemaphore ops placed immediately before it (engine
    queues execute in order, so semantics are preserved)."""
    for f in nc.m.functions:
        for blk in f.blocks:
            new_insts = []
            changed = False
            for inst in blk.instructions:
                si = inst.sync_info
                if si is not None and len(si.on_wait) > 1:
                    waits = list(si.on_wait)
                    for k, w in enumerate(waits[:-1]):
                        new_insts.append(mybir.InstEventSemaphore(
                            name=f"{inst.name}-sw{k}",
                            engine=inst.engine,
                            ins=[], outs=[],
                            sync_info=mybir.SyncInfo(on_wait=[w], on_update=[]),
                        ))
                    inst.sync_info = mybir.SyncInfo(
                        on_wait=[waits[-1]], on_update=list(si.on_update))
                    changed = True
                new_insts.append(inst)
            if changed:
                blk.instructions = new_insts
    return nc


def build_program(t_steps=T, d_dim=D, split_waits=True, scan_reps=1,
                  loop_reps=None, warm=0, mode=None):
    """mode: 'base' = previous kernel; 'a' = DVE stt logit-add (no ACT);
    'ab' = 'a' + K=32 tile-position-concurrent PE matmuls for the
    broadcast/trans rounds."""
    if mode is None:
        mode = os.environ.get("CRF_MODE", "base")
    nt = t_steps * BL                       # columns in (t, b) layout
    ch = min(512, nt)                       # DMA/matmul chunk width
    nch = nt // ch
    kblocks = d_dim // 128

    nc = bass.Bass(trn_type="TRN2")

    xdt = nc.dram_tensor("xdt", [d_dim, nt], F32, kind="ExternalInput")
    ker = nc.dram_tensor("ker", [d_dim, U], F32, kind="ExternalInput")
    translhs = nc.dram_tensor("translhs", [U, U], F32, kind="ExternalInput")
    ident = nc.dram_tensor("ident", [U, U], F32, kind="ExternalInput")
    ident32 = nc.dram_tensor("ident32", [U, 32], F32, kind="ExternalInput")
    lbv = nc.dram_tensor("lbv", [U, 1], F32, kind="ExternalInput")
    rbv = nc.dram_tensor("rbv", [U, 1], F32, kind="ExternalInput")
    biasrow = nc.dram_tensor("biasrow", [1, U], F32, kind="ExternalInput")
    onesrow = nc.dram_tensor("onesrow", [1, 512], F32, kind="ExternalInput")
    vout = nc.dram_tensor("vout", [U, nt], F32, kind="ExternalOutput")

    with TileContext(nc) as tc:
        with (
            tc.tile_pool(name="const", bufs=1) as cpool,
            tc.tile_pool(name="xp", bufs=10) as xpool,
            tc.tile_pool(name="big", bufs=1) as bigpool,
            tc.tile_pool(name="mx", bufs=3) as mxpool,
            tc.tile_pool(name="mmps", bufs=2, space="PSUM") as mmpool,
            tc.tile_pool(name="scps", bufs=2, space="PSUM") as scpool,
            tc.tile_pool(name="wmps", bufs=1, space="PSUM") as wmpool,
        ):
            # ---- constants into SBUF ----
            ker_sb = []
            for kb in range(kblocks):
                kt = cpool.tile([128, U], F32, tag=f"ker{kb}")
                nc.sync.dma_start(out=kt[:, :], in_=ker[kb * 128:(kb + 1) * 128, :])
                ker_sb.append(kt)
            trans_sb = cpool.tile([U, U], F32, tag="trans")
            nc.sync.dma_start(out=trans_sb[:, :], in_=translhs[:, :])
            ident_sb = cpool.tile([U, U], F32, tag="ident")
            nc.sync.dma_start(out=ident_sb[:, :], in_=ident[:, :])
            ident32_sb = cpool.tile([U, 32], F32, tag="ident32")
            nc.sync.dma_start(out=ident32_sb[:, :], in_=ident32[:, :])
            lb_sb = cpool.tile([U, 1], F32, tag="lb")
            nc.sync.dma_start(out=lb_sb[:, :], in_=lbv[:, :])
            rb_sb = cpool.tile([U, 1], F32, tag="rb")
            nc.sync.dma_start(out=rb_sb[:, :], in_=rbv[:, :])
            biasrow_sb = cpool.tile([1, U], F32, tag="biasrow")
            nc.sync.dma_start(out=biasrow_sb[:, :], in_=biasrow[:, :])
            onesrow_sb = cpool.tile([1, 512], F32, tag="onesrow")
            nc.sync.dma_start(out=onesrow_sb[:, :], in_=onesrow[:, :])

            logitsT = bigpool.tile([U, nt], F32, tag="logitsT")
            # per-group v history (decoupled so the two batch-group
            # pipelines never serialize on tile dependency tracking);
            # group g columns: t * GB + bb
            vh = [bigpool.tile([U, nt // NG], F32, tag=f"vh{g}",
                                name=f"vh{g}")
                  for g in range(NG)]

            # ---- phase 1: logits = kernel.T @ x (+bias) ----
            for c in range(nch):
                ps = mmpool.tile([128, ch], F32, tag="mm")
                for kb in range(kblocks):
                    xt = xpool.tile([128, ch], F32, tag="x")
                    nc.sync.dma_start(
                        out=xt[:, :],
                        in_=xdt[kb * 128:(kb + 1) * 128, c * ch:(c + 1) * ch],
                    )
                    nc.tensor.matmul(
                        out=ps[:, :], lhsT=ker_sb[kb][:, :], rhs=xt[:, :],
                        start=(kb == 0), stop=False,
                    )
                nc.tensor.matmul(
                    out=ps[:, :], lhsT=biasrow_sb[0:1, :],
                    rhs=onesrow_sb[0:1, 0:ch], start=False, stop=True,
                )
                nc.scalar.copy(
                    out=logitsT[:, c * ch:(c + 1) * ch], in_=ps[:, :],
                )

            # right boundary folded into the last timestep's logits
            nc.vector.tensor_scalar_add(
                out=logitsT[:, (t_steps - 1) * BL:],
                in0=logitsT[:, (t_steps - 1) * BL:],
                scalar1=rb_sb[:, 0:1],
            )
            # ---- phase 2: Viterbi forward scan ----
            # v broadcast across partitions: matmul with the previous v
            # column as stationary operand, broadcast along its free dim
            # (step-0 AP), against an identity moving operand:
            #   out[p, i] = sum_k v[k] * I[k, i] = v[i]  for every p.
            # scan_reps > 1 repeats the whole scan (for differential
            # wall-clock timing); results are identical each rep.
            steps_per_chunk = ch // BL
            gch = steps_per_chunk * GB          # per-group chunk width
            import contextlib
            rep_ctx = (tc.For_i(0, loop_reps, 1) if loop_reps
                       else contextlib.nullcontext())
            if mode.startswith("c"):
                # ---- mode c: 6 PE/DVE batches (2 groups of 3) + 2 ACT/Pool
                # batches. vout layout: [vhE0 (3T) | vhE1 (3T) | vhA (2T)].
                import concourse.bass_isa as bass_isa
                NEG, EGB = 2, 3                   # E groups, batches/group
                vhE = [bigpool.tile([U, t_steps * EGB], F32, tag=f"vhe{g}",
                                    name=f"vhe{g}") for g in range(NEG)]
                vhA = bigpool.tile([U, t_steps * 2], F32, tag="vha",
                                   name="vha")
                areduce = os.environ.get("CRF_AREDUCE", "pallred")
                if areduce == "pallred":
                    from concourse import library_config
                    nc.gpsimd.load_library(library_config.mlp)
                with (
                    tc.tile_pool(name="sa", bufs=2) as sapool,
                    tc.tile_pool(name="ma", bufs=2) as mapool,
                    tc.tile_pool(name="mc", bufs=2, space="PSUM") as mcpool,
                ):
                  with rep_ctx:
                    for _rep in range(scan_reps):
                      for g in range(NEG):
                          nc.vector.tensor_scalar_add(
                              out=vhE[g][:, 0:EGB],
                              in0=logitsT[:, g * EGB:(g + 1) * EGB],
                              scalar1=lb_sb[:, 0:1])
                      nc.vector.tensor_scalar_add(
                          out=vhA[:, 0:2], in0=logitsT[:, 6:8],
                          scalar1=lb_sb[:, 0:1])
                      for t in range(1, t_steps):
                          for g in range(NEG):
                              lcols0 = t * BL + g * EGB
                              vcols0 = t * EGB
                              pcol0 = (t - 1) * EGB
                              sc = scpool.tile([128, EGB * U], F32,
                                               tag=f"sc{g}")
                              if mode == "cT":
                                  for bb in range(EGB):
                                      vcol = vhE[g][:, pcol0 + bb:
                                                    pcol0 + bb + 1]
                                      nc.tensor.matmul(
                                          out=sc[:, bb * U:(bb + 1) * U],
                                          lhsT=vcol.broadcast_to([U, U]),
                                          rhs=ident_sb[:, :],
                                          start=(bb == 0), stop=False,
                                          skip_group_check=True,
                                          is_transpose=True)
                                  for bb in range(EGB):
                                      nc.tensor.matmul(
                                          out=sc[:, bb * U:(bb + 1) * U],
                                          lhsT=trans_sb[:, :],
                                          rhs=ident_sb[:, :],
                                          start=False, stop=(bb == EGB - 1),
                                          skip_group_check=True,
                                          is_transpose=True)
                              else:
                                  for bb in range(EGB):
                                      vcol = vhE[g][:, pcol0 + bb:
                                                    pcol0 + bb + 1]
                                      for blk in range(4):
                                          nc.tensor.matmul(
                                              out=sc[:, bb * U + 32 * blk:
                                                     bb * U + 32 * blk + 32],
                                              lhsT=vcol[32 * blk:
                                                        32 * blk + 32, :]
                                                  .broadcast_to([32, U]),
                                              rhs=ident32_sb[32 * blk:
                                                             32 * blk + 32, :],
                                              start=(bb == 0 and blk == 0),
                                              stop=False,
                                              skip_group_check=True,
                                              tile_position=(32 * blk, 0))
                                  for bb in range(EGB):
                                      for blk in range(4):
                                          nc.tensor.matmul(
                                              out=sc[:, bb * U + 32 * blk:
                                                     bb * U + 32 * blk + 32],
                                              lhsT=trans_sb[32 * blk:
                                                            32 * blk + 32, :],
                                              rhs=ident32_sb[32 * blk:
                                                             32 * blk + 32, :],
                                              start=False,
                                              stop=(bb == EGB - 1 and
                                                    blk == 3),
                                              skip_group_check=True,
                                              tile_position=(32 * blk, 0))
                              mx = mxpool.tile([U, EGB], F32, tag=f"mx{g}")
                              nc.vector.tensor_reduce(
                                  out=mx[:, :],
                                  in_=sc[:, :].rearrange("p (b i) -> p b i",
                                                         i=U),
                                  axis=mybir.AxisListType.X,
                                  op=mybir.AluOpType.max)
                              nc.vector.tensor_add(
                                  out=vhE[g][:, vcols0:vcols0 + EGB],
                                  in0=mx[:, :],
                                  in1=logitsT[:, lcols0:lcols0 + EGB])
                          # A route: scores[i, j] built by ACT (bias = prev
                          # v col), partition-max on Pool, PE transposes the
                          # result row back to a column, DVE adds the logit.
                          mc = mcpool.tile([U, 2], F32, tag="mc")
                          for ai in range(2):
                              sa = sapool.tile([U, U], F32, tag=f"sa{ai}")
                              nc.scalar.activation(
                                  out=sa[:, :], in_=trans_sb[:, :],
                                  func=mybir.ActivationFunctionType.Identity,
                                  bias=vhA[:, (t - 1) * 2 + ai:
                                           (t - 1) * 2 + ai + 1])
                              if areduce == "pallred":
                                  ma = mapool.tile([U, U], F32, tag=f"ma{ai}")
                                  nc.gpsimd.partition_all_reduce(
                                      out_ap=ma[:, :], in_ap=sa[:, :],
                                      channels=U,
                                      reduce_op=bass_isa.ReduceOp.max)
                                  marow = ma[0:1, :]
                              else:
                                  ma = mapool.tile([1, U], F32, tag=f"ma{ai}")
                                  nc.gpsimd.tensor_reduce(
                                      out=ma[:, :], in_=sa[:, :],
                                      axis=mybir.AxisListType.C,
                                      op=mybir.AluOpType.max)
                                  marow = ma[:, :]
                              nc.tensor.matmul(
                                  out=mc[:, ai:ai + 1], lhsT=marow,
                                  rhs=ident_sb[0:1, 0:1], start=True,
                                  stop=True, skip_group_check=True,
                                  is_transpose=True)
                          nc.vector.tensor_add(
                              out=vhA[:, t * 2:t * 2 + 2], in0=mc[:, 0:2],
                              in1=logitsT[:, t * BL + 6:t * BL + 8])
                          if (t + 1) % steps_per_chunk == 0:
                              cc = (t + 1) // steps_per_chunk - 1
                              spc = steps_per_chunk
                              for g in range(NEG):
                                  nc.sync.dma_start(
                                      out=vout[:, g * (t_steps * EGB) +
                                               cc * spc * EGB:
                                               g * (t_steps * EGB) +
                                               (cc + 1) * spc * EGB],
                                      in_=vhE[g][:, cc * spc * EGB:
                                                 (cc + 1) * spc * EGB])
                              nc.sync.dma_start(
                                  out=vout[:, 6 * t_steps + cc * spc * 2:
                                           6 * t_steps + (cc + 1) * spc * 2],
                                  in_=vhA[:, cc * spc * 2:(cc + 1) * spc * 2])
            if mode.startswith("c"):
                # c-scan already emitted above; skip the default scan body
                rep_ctx = contextlib.nullcontext()
                scan_iter = []
            else:
                scan_iter = range(scan_reps)
            with rep_ctx:
             for _rep in scan_iter:
              # v_0 = logits_0 + left boundary
              for g in range(NG):
                nc.vector.tensor_scalar_add(
                    out=vh[g][:, 0:GB], in0=logitsT[:, g * GB:(g + 1) * GB],
                    scalar1=lb_sb[:, 0:1],
                )
              for t in range(1, t_steps):
                  for g in range(NG):
                      lcols0 = t * BL + g * GB    # logitsT columns
                      vcols0 = t * GB             # vh[g] columns
                      pcol0 = (t - 1) * GB
                      sc = scpool.tile([128, GB * U], F32, tag=f"sc{g}")
                      if mode in ("b", "ab", "a2b", "a3b"):
                          # scores via K=32 row-group-tiled matmuls: the 4
                          # tiles of a round occupy distinct PE row groups
                          # (tile_position) and run concurrently. Broadcast
                          # round: out[p, i in blk] = sum_k v[32B+k] I[k,i]
                          # = v[i]; trans round accumulates trans[i, p].
                          # start=True only on the tile's very first matmul:
                          # start clears has_written for the whole PSUM bank,
                          # so a later start would orphan earlier blocks'
                          # writes (their accumulation step then overwrites).
                          for bb in range(GB):
                              vcol = vh[g][:, pcol0 + bb:pcol0 + bb + 1]
                              for blk in range(4):
                                  nc.tensor.matmul(
                                      out=sc[:, bb * U + 32 * blk:
                                             bb * U + 32 * blk + 32],
                                      lhsT=vcol[32 * blk:32 * blk + 32, :]
                                          .broadcast_to([32, U]),
                                      rhs=ident32_sb[32 * blk:32 * blk + 32, :],
                                      start=(bb == 0 and blk == 0), stop=False,
                                      skip_group_check=True,
                                      tile_position=(32 * blk, 0),
                                  )
                          for bb in range(GB):
                              for blk in range(4):
                                  nc.tensor.matmul(
                                      out=sc[:, bb * U + 32 * blk:
                                             bb * U + 32 * blk + 32],
                                      lhsT=trans_sb[32 * blk:32 * blk + 32, :],
                                      rhs=ident32_sb[32 * blk:32 * blk + 32, :],
                                      start=False,
                                      stop=(bb == GB - 1 and blk == 3),
                                      skip_group_check=True,
                                      tile_position=(32 * blk, 0),
                                  )
                      else:
                          # mode "r": float32r transposes run 1.5 cyc/row on
                          # the PE vs 2.0 for fp32 — a pure-data-movement
                          # bitcast, no arithmetic change (verified exact).
                          r32 = mybir.dt.float32r
                          cast = ((lambda ap: ap.bitcast(r32))
                                  if mode == "r" else (lambda ap: ap))
                          for bb in range(GB):
                              vcol = vh[g][:, pcol0 + bb:pcol0 + bb + 1]
                              nc.tensor.matmul(
                                  out=cast(sc[:, bb * U:(bb + 1) * U]),
                                  lhsT=cast(vcol.broadcast_to([U, U])),
                                  rhs=cast(ident_sb[:, :]),
                                  start=(bb == 0), stop=False,
                                  skip_group_check=True, is_transpose=True,
                              )
                          for bb in range(GB):
                              nc.tensor.matmul(
                                  out=cast(sc[:, bb * U:(bb + 1) * U]),
                                  lhsT=cast(trans_sb[:, :]),
                                  rhs=cast(ident_sb[:, :]),
                                  start=False, stop=(bb == GB - 1),
                                  skip_group_check=True, is_transpose=True,
                              )
                      # optional HAM-warming filler: keeps the PE p-state
                      # hot across the per-step stall waiting for v(t)
                      for _w in range(warm):
                          wt = wmpool.tile([U, U], F32, tag="warm")
                          nc.tensor.matmul(
                              out=wt[:, :], lhsT=ident_sb[:, :],
                              rhs=ident_sb[:, :], start=True, stop=True,
                              skip_group_check=True, is_transpose=True,
                          )
                      mx = mxpool.tile([U, GB], F32, tag=f"mx{g}")
                      nc.vector.tensor_reduce(
                          out=mx[:, :],
                          in_=sc[:, :].rearrange("p (b i) -> p b i", i=U),
                          axis=mybir.AxisListType.X, op=mybir.AluOpType.max,
                      )
                      if mode in ("a", "ab"):
                          # v = mx + logit in one grouped DVE op (the ACT
                          # per-column path costs ~190 ns/instruction)
                          nc.vector.scalar_tensor_tensor(
                              out=vh[g][:, vcols0:vcols0 + GB],
                              in0=mx[:, :], scalar=1.0,
                              in1=logitsT[:, lcols0:lcols0 + GB],
                              op0=mybir.AluOpType.mult,
                              op1=mybir.AluOpType.add,
                          )
                      elif mode in ("a2", "a2b"):
                          nc.vector.tensor_add(
                              out=vh[g][:, vcols0:vcols0 + GB],
                              in0=mx[:, :],
                              in1=logitsT[:, lcols0:lcols0 + GB],
                          )
                      elif mode in ("a3", "a3b"):
                          # logit add on the otherwise-idle Pool engine
                          nc.gpsimd.tensor_add(
                              out=vh[g][:, vcols0:vcols0 + GB],
                              in0=mx[:, :],
                              in1=logitsT[:, lcols0:lcols0 + GB],
                          )
                      else:
                          for bb in range(GB):
                              nc.scalar.activation(
                                  out=vh[g][:, vcols0 + bb:vcols0 + bb + 1],
                                  in_=mx[:, bb:bb + 1],
                                  func=mybir.ActivationFunctionType.Identity,
                                  bias=logitsT[:, lcols0 + bb:lcols0 + bb + 1],
                              )
                  if (t + 1) % steps_per_chunk == 0:
                      c = (t + 1) // steps_per_chunk - 1
                      for g in range(NG):
                          nc.sync.dma_start(
                              out=vout[:, g * (nt // NG) + c * gch:
                                       g * (nt // NG) + (c + 1) * gch],
                              in_=vh[g][:, c * gch:(c + 1) * gch],
                          )
    return split_multi_waits(nc) if split_waits else nc


def make_in_map(x_core, ker, bias, trans, lb, rb, t_steps=T, d_dim=D):
    """x_core: [BL, t_steps, d_dim] float32."""
    nt = t_steps * BL
    xdt = np.ascontiguousarray(x_core.transpose(2, 1, 0)).reshape(d_dim, nt)
    return {
        "xdt": xdt.astype(np.float32),
        "ker": np.ascontiguousarray(ker, dtype=np.float32),
        "biasrow": np.ascontiguousarray(bias, dtype=np.float32).reshape(1, U),
        "onesrow": np.ones((1, 512), dtype=np.float32),
        "translhs": np.ascontiguousarray(trans, dtype=np.float32),
        "ident": np.eye(U, dtype=np.float32),
        "ident32": np.tile(np.eye(32, dtype=np.float32), (U // 32, 1)),
        "lbv": np.ascontiguousarray(lb, dtype=np.float32).reshape(U, 1),
        "rbv": np.ascontiguousarray(rb, dtype=np.float32).reshape(U, 1),
    }


def backtrace(v, trans):
    """v: [b, t, u] forward max values; trans: [u, u]. Returns int tags [b, t]."""
    nb, nt, nu = v.shape
    tags = np.zeros((nb, nt), dtype=np.int64)
    cur = np.argmax(v[:, -1, :], axis=1)
    tags[:, -1] = cur
    for t in range(nt - 2, -1, -1):
        scores = v[:, t, :] + trans[:, cur].T     # fp32, same as device order
        cur = np.argmax(scores, axis=1)
        tags[:, t] = cur
    return tags


def vout_to_v(vout_core, t_steps=T, mode=None):
    """Decode vout -> v [BL, t, U] for the given mode's layout."""
    if mode is None:
        mode = os.environ.get("CRF_MODE", "base")
    if mode.startswith("c"):
        # [vhE0 (t*3+bb) | vhE1 | vhA (t*2+ai)]
        v = np.empty((BL, t_steps, U), np.float32)
        e0 = vout_core[:, 0:3 * t_steps].reshape(U, t_steps, 3)
        e1 = vout_core[:, 3 * t_steps:6 * t_steps].reshape(U, t_steps, 3)
        av = vout_core[:, 6 * t_steps:8 * t_steps].reshape(U, t_steps, 2)
        v[0:3] = e0.transpose(2, 1, 0)
        v[3:6] = e1.transpose(2, 1, 0)
        v[6:8] = av.transpose(2, 1, 0)
        return v
    v = vout_core.reshape(U, NG, t_steps, GB)     # [u, g, t, bb]
    return np.ascontiguousarray(v.transpose(1, 3, 2, 0).reshape(BL, t_steps, U))


def kernel(x, kernel, bias, chain_kernel, left_boundary, right_boundary):
    x = np.asarray(x, dtype=np.float32)
    ker = np.asarray(kernel, dtype=np.float32)
    bias = np.asarray(bias, dtype=np.float32)
    trans = np.asarray(chain_kernel, dtype=np.float32)
    lb = np.asarray(left_boundary, dtype=np.float32)
    rb = np.asarray(right_boundary, dtype=np.float32)

    nc = build_program()
    in_maps = [
        make_in_map(x[c * BL:(c + 1) * BL], ker, bias, trans, lb, rb)
        for c in range(NCORES)
    ]
    kwargs = {}
    if os.environ.get("CRF_TRACE"):
        kwargs = {"trace": True, "tmpdir": os.environ.get("CRF_TRACE_DIR") or None}
    res = run_bass_kernel_spmd(nc, in_maps, core_ids=list(range(NCORES)), **kwargs)
    global last_results
    last_results = res
    v = np.concatenate(
        [vout_to_v(np.asarray(r["vout"])) for r in res.results], axis=0)
    tags = backtrace(v, trans)
    return tags.astype(np.float32)

